# revision 17
# baseline (speedup 1.0000x reference)
"""Trainium2 Bass kernel for causal GQA self-attention (B=2, S=2048, H=2048,
16 heads / 4 KV heads, head_dim 128) on 8 NeuronCores.

Sharding: core i = (batch b=i//4, kv-group g=i%4). Each core computes the QKV
projections for its batch restricted to its group's channels (q heads 4g..4g+3,
kv head g), attention for those 4 heads over its batch's 2048 rows, then a
8-rank AllToAll per head switches to row-sharding: core (b,g) computes rows
256*i of both batches through the full o_proj.

Per-core dataflow (channel-major layouts so no transposes of big activations):
  XT[h, r] (bf16, host-pretransposed) --matmul--> QT/KT/VT; V via
  dma_start_transpose of VT tiles; scores S^T[k, q] = KT_tile.T @ QT; causal
  handled by compile-time tile skipping + one [128,128] triangle mask; exp on
  ScalarE (scale folded in, every 2nd non-diag tile via DVE fast-exp bit
  trick); AV accumulated in PSUM over k tiles with a TWO-deep software
  pipeline interleaved across (h,qb) blocks so exp latency never bubbles the
  PE; softmax denominators via DVE quad-merge of exp tiles (3 adds per 4
  tiles) + ONE ones-matmul per quad (4x fewer PE colsum rows than per-tile);
  normalize with reciprocal_approx_fast + mul; per-head A2A (at-tile loads
  ride the scalar ring so they can never block the next head's ao writes);
  o_proj from SBUF-resident Wo, pass-1 (heads 0-2) results parked in SBUF
  bf16, pass-2 (head 3) added on DVE and written out once on the sync ring
  (no read-modify-write DMA tail).

DMA plan: xt stream entirely on sync ring with 6-chunk prefetch depth (the
ring sustains ~150GB/s when kept fed; splitting across rings let it idle);
QKV weights on scalar; consts + Wo bulk + collective triggers on gpsimd
(SWDGE hits 130-200GB/s for the 8MB Wo preload).
"""

import sys

sys.path.insert(0, "/opt/trn_rl_repo")

from collections import deque
from contextlib import ExitStack

import numpy as np
import ml_dtypes

import concourse.bass as bass
import concourse.mybir as mybir
import concourse.tile as tile
from concourse import bacc
from concourse.bass_utils import run_bass_kernel_spmd

F32 = mybir.dt.float32
BF16 = mybir.dt.bfloat16
I16 = mybir.dt.int16
AF = mybir.ActivationFunctionType

N_CORES = 8
B, S, HID = 2, 2048, 2048
NH, NKV, D = 16, 4, 128
SCALE = 1.0 / np.sqrt(D)
NEG = -1e30
P = 128
N_KT = HID // P  # 16 contraction tiles
N_RB = S // 512  # 4 row blocks (one batch per core)

import os
WO_PRELOAD = os.environ.get("K_WO_PRELOAD", "1") == "1"
# Every K-th non-diagonal exp tile runs as a 1-op DVE fast-exp (Schraudolph
# bf16 bit trick, ~2% rms) instead of on ScalarE; 0 disables.
FASTEXP_K = int(os.environ.get("K_FASTEXP", "3"))
# colsum via DVE quad-merge (1 ones-matmul per 4 exp tiles) vs per-tile
QUAD_CS = os.environ.get("K_QUAD", "1") == "1"
# AV software-pipeline depth (k-tiles the AV matmul trails the scores matmul)
PEND_DEPTH = int(os.environ.get("K_PEND", "6"))
# pass-1 o_proj parked in SBUF, pass-2 added on DVE (vs accum-DMA)
YSB = os.environ.get("K_YSB", "1") == "1"
LN2 = float(np.log(2.0))
FE_A = float(2.0**7 / LN2) * SCALE
FE_B = float(127.0 * 128.0 - 5.5)


def build_nc(debug=False):
    nc = bacc.Bacc("TRN2", target_bir_lowering=False, debug=debug, num_devices=8)

    # All big inputs are host-pre-permuted into their exact on-chip layouts so
    # every DMA is long-contiguous.
    xt = nc.dram_tensor("xt", [N_RB, P, N_KT, 512], BF16, kind="ExternalInput")
    wq = nc.dram_tensor("wq", [P, N_KT, 512], BF16, kind="ExternalInput")
    wk = nc.dram_tensor("wk", [P, N_KT, 128], BF16, kind="ExternalInput")
    wv = nc.dram_tensor("wv", [P, N_KT, 128], BF16, kind="ExternalInput")
    bq = nc.dram_tensor("bq", [512, 1], F32, kind="ExternalInput")
    bk = nc.dram_tensor("bk", [128, 1], F32, kind="ExternalInput")
    bv = nc.dram_tensor("bv", [128, 1], F32, kind="ExternalInput")
    wo = nc.dram_tensor("wo", [P, N_KT, HID], BF16, kind="ExternalInput")
    bo = nc.dram_tensor("bo", [1, HID], BF16, kind="ExternalInput")
    mtri = nc.dram_tensor("mtri", [P, P], F32, kind="ExternalInput")
    onesd = nc.dram_tensor("onesd", [P, P], BF16, kind="ExternalInput")
    y = nc.dram_tensor("y", [4, 4, P, 512], BF16, kind="ExternalOutput")

    with tile.TileContext(nc) as tc, ExitStack() as top:
        persist = top.enter_context(tc.tile_pool(name="persist", bufs=1))
        dram = top.enter_context(tc.tile_pool(name="dram", bufs=1, space="DRAM"))

        a2a_in = [dram.tile([8, P, 256], BF16, name=f"a2a_in{h}") for h in range(4)]
        a2a_out = [dram.tile([8, P, 256], BF16, name=f"a2a_out{h}") for h in range(4)]

        # Ring plan: sync = xt stream + ao writes + final y writes;
        # scalar = QKV weights + V transposes + at loads;
        # gpsimd = consts + Wo bulk + collective triggers.
        wq_sb = persist.tile([P, N_KT, 512], BF16, tag="wq")
        wk_sb = persist.tile([P, N_KT, 128], BF16, tag="wk")
        wv_sb = persist.tile([P, N_KT, 128], BF16, tag="wv")

        def load_w_chunk(kc):
            tsl = slice(4 * kc, 4 * (kc + 1))
            nc.scalar.dma_start(wq_sb[:, tsl, :], wq[:, tsl, :])
            nc.scalar.dma_start(wk_sb[:, tsl, :], wk[:, tsl, :])
            nc.scalar.dma_start(wv_sb[:, tsl, :], wv[:, tsl, :])

        load_w_chunk(0)

        ones_sq = persist.tile([P, P], BF16, tag="ones_sq")
        nc.gpsimd.dma_start(ones_sq[:], onesd[:])
        mtri_sb = persist.tile([P, P], F32, tag="mtri")
        nc.gpsimd.dma_start(mtri_sb[:], mtri[:])
        bq_sb = persist.tile([P, 4], F32, tag="bq")
        for h in range(4):
            nc.gpsimd.dma_start(bq_sb[:, h : h + 1], bq[128 * h : 128 * (h + 1), :])
        bk_sb = persist.tile([P, 1], F32, tag="bk")
        nc.gpsimd.dma_start(bk_sb[:], bk[:])
        bv_sb = persist.tile([P, 1], F32, tag="bv")
        nc.gpsimd.dma_start(bv_sb[:], bv[:])
        bo_sb = persist.tile([1, HID], BF16, tag="bo")
        nc.gpsimd.dma_start(bo_sb[:], bo[:])

        # Full Wo lives in SBUF: streamed on the gpsimd (SWDGE) ring, gated to
        # start only mid-phase-1 (see wo_gate below) so it can't starve the xt
        # stream; o_proj is then pure compute.
        wo_sb = persist.tile([P, N_KT, HID], BF16, tag="wo")

        # channel-major activations: partitions = feature dim
        qt_sb = [persist.tile([P, S], BF16, tag=f"qt{h}", name=f"qt{h}") for h in range(4)]
        kt_sb = persist.tile([P, S], BF16, tag="kt")
        v_sb = persist.tile([P, N_RB * 4, P], BF16, tag="v")  # [krow%128, ktile, d]
        vt_full = persist.tile([P, N_RB, 512], BF16, tag="vtf")
        if YSB:
            y_sb = [persist.tile([P, HID], BF16, tag=f"ysb{r}", name=f"ysb{r}") for r in range(4)]

        # ---- Phase 1: QKV projections (+ V transpose via DMA XBAR) ----
        with ExitStack() as ph1:
            xpool = ph1.enter_context(tc.tile_pool(name="xp", bufs=6))
            pspool = ph1.enter_context(tc.tile_pool(name="ps1", bufs=8, space="PSUM"))
            for rb in range(N_RB):
                rsl = slice(512 * rb, 512 * (rb + 1))
                # 4 chunks of 4 k-tiles each, all on the sync ring (weights own
                # the scalar ring); 6-buf pool keeps ~3 chunks prefetched ahead
                xt_c = [xpool.tile([P, 4, 512], BF16, tag="x", name=f"xt{rb}_{kc}") for kc in range(4)]
                for kc in range(4):
                    nc.sync.dma_start(xt_c[kc][:], xt[rb, :, 4 * kc : 4 * (kc + 1), :])
                    if rb == 0 and kc == 0:
                        load_w_chunk(1)
                    elif rb == 0 and kc == 2:
                        load_w_chunk(2)
                        load_w_chunk(3)
                if rb == 1 and WO_PRELOAD:
                    # gate the Wo stream on rb0's V evac, then emit its loads:
                    # they start once phase 1 is established and can't starve
                    # the early xt chunks.
                    wo_gate = persist.tile([1, 1], BF16, tag="wog")
                    nc.gpsimd.tensor_copy(wo_gate[:], vt_full[0:1, 0, 0:1])
                    for t in range(N_KT):
                        nc.gpsimd.dma_start(wo_sb[:, t, :], wo[:, t, :])
                ps_q = [pspool.tile([P, 512], F32, tag="ps1", name=f"ps_q{h}") for h in range(4)]
                ps_k = pspool.tile([P, 512], F32, tag="ps1", name="ps_k")
                ps_v = pspool.tile([P, 512], F32, tag="ps1", name="ps_v")
                for kt_i in range(N_KT):
                    st, sp = kt_i == 0, kt_i == N_KT - 1
                    x_sl = xt_c[kt_i // 4][:, kt_i % 4, :]
                    for h in range(4):
                        nc.tensor.matmul(
                            ps_q[h][:], wq_sb[:, kt_i, 128 * h : 128 * (h + 1)],
                            x_sl, start=st, stop=sp,
                        )
                    nc.tensor.matmul(ps_k[:], wk_sb[:, kt_i, :], x_sl, start=st, stop=sp)
                    nc.tensor.matmul(ps_v[:], wv_sb[:, kt_i, :], x_sl, start=st, stop=sp)
                for h in range(4):
                    nc.vector.tensor_scalar_add(qt_sb[h][:, rsl], ps_q[h][:], bq_sb[:, h : h + 1])
                nc.vector.tensor_scalar_add(kt_sb[:, rsl], ps_k[:], bk_sb[:])
                nc.vector.tensor_scalar_add(vt_full[:, rb, :], ps_v[:], bv_sb[:])
                # V transposes per rb, right after its evac: they interleave
                # behind the remaining weight chunks on the scalar ring and
                # finish before phase 2's first AV needs v_sb (instead of
                # all queueing in the phase-1 tail)
                for j in range(4):
                    m = 4 * rb + j
                    nc.scalar.dma_start_transpose(
                        v_sb[:, m, :], vt_full[:, rb, P * j : P * (j + 1)]
                    )

        # ---- Phase 2: attention (flash-style, S^T layout), A2A per head ----
        with ExitStack() as ph2:
            espool = ph2.enter_context(tc.tile_pool(name="es", bufs=PEND_DEPTH + 4))
            qapool = ph2.enter_context(tc.tile_pool(name="qa", bufs=2))
            bcpool = ph2.enter_context(tc.tile_pool(name="bc", bufs=2))
            aopool = ph2.enter_context(tc.tile_pool(name="ao", bufs=2))
            pss = ph2.enter_context(tc.tile_pool(name="pss", bufs=4, space="PSUM"))
            psav = ph2.enter_context(tc.tile_pool(name="psav", bufs=2, space="PSUM"))
            pscs = ph2.enter_context(tc.tile_pool(name="pscs", bufs=2, space="PSUM"))
            at = [
                [
                    persist.tile([P, 256], BF16, tag=f"at{bb}_{t}", name=f"at{bb}_{t}")
                    for t in range(N_KT)
                ]
                for bb in range(B)
            ]
            ndiag_ctr = 0
            # AV matmuls trail the scores matmuls by PEND_DEPTH k-tiles,
            # across (h,qb) block boundaries, so the exp of a block's last
            # tile overlaps the next block's first scores matmuls on the PE.
            # A block's normalization (recip/mul/ao DMA) and - for qb==3 -
            # its head's A2A trigger are emitted right after its final AV
            # flushes, keeping program order consistent with dataflow while
            # never parking the DVE on a not-yet-emitted matmul.
            pend = deque()

            def finalize_block(h, qb, ps_av, ps_cs):
                bc = bcpool.tile([P, 512], F32, tag="bc", name="bc")
                nc.vector.reciprocal_approx_fast(bc[:], ps_cs[:])
                ao = aopool.tile([P, 512], BF16, tag="ao", name="ao")
                nc.vector.tensor_mul(ao[:], ps_av[:], bc[:])
                nc.sync.dma_start(a2a_in[h][2 * qb, :, :], ao[:, 0:256])
                nc.sync.dma_start(a2a_in[h][2 * qb + 1, :, :], ao[:, 256:512])
                if qb == 3:
                    # ---- per-head 8-rank AllToAll; overlaps the next head's
                    # compute. Slot j carries my head's output for the 256
                    # rows of my batch that core j o_proj-owns.
                    nc.gpsimd.collective_compute(
                        "AllToAll",
                        mybir.AluOpType.bypass,
                        replica_groups=[list(range(N_CORES))],
                        ins=[a2a_in[h][:]],
                        outs=[a2a_out[h][:]],
                    )
                    # at-tile loads ride the GPSIMD ring: a dma_start's
                    # semaphore wait blocks the ISSUING engine, and gpsimd
                    # only carries cc triggers here - cc h+1 cannot start
                    # before cc h completes anyway (serial cc stream), so
                    # parking gpsimd on A2A-h completion costs nothing,
                    # while on sync/scalar it starved ao writes / exps.
                    for i in range(N_CORES):
                        bb, g = i // 4, i % 4
                        t = 4 * g + h
                        nc.gpsimd.dma_start(at[bb][t][:], a2a_out[h][i, :, :])

            def flush_pend():
                ki, q0, es, blk, st, sp = pend.popleft()
                h, qb, ps_av, ps_cs = blk
                nc.tensor.matmul(
                    ps_av[:, q0:512], v_sb[:, ki, :],
                    es[:, q0:512], start=st, stop=sp,
                    skip_group_check=True,
                )
                if not QUAD_CS:
                    nc.tensor.matmul(
                        ps_cs[:, q0:512], ones_sq[:],
                        es[:, q0:512], start=st, stop=sp,
                        skip_group_check=True,
                    )
                if sp:
                    finalize_block(h, qb, ps_av, ps_cs)

            for h in range(4):
                for qb in range(4):
                    # diagonal k-tiles first (full q width on the first)
                    ktiles = list(range(4 * qb, 4 * qb + 4)) + list(range(4 * qb))
                    ps_av = psav.tile([P, 512], F32, tag="av", name="ps_av")
                    ps_cs = pscs.tile([P, 512], F32, tag="cs", name="ps_cs")
                    blk = (h, qb, ps_av, ps_cs)
                    n_kt_q = len(ktiles)
                    n_quad = n_kt_q // 4
                    qa = None
                    for idx, ki in enumerate(ktiles):
                        diag = ki >= 4 * qb
                        q0 = 128 * ki - 512 * qb if diag else 0
                        ps_s = pss.tile([P, 512], F32, tag="s", name="ps_s")
                        ksl = kt_sb[:, P * ki : P * (ki + 1)]
                        qsl = qt_sb[h][:, 512 * qb + q0 : 512 * (qb + 1)]
                        nc.tensor.matmul(
                            ps_s[:, q0:512], ksl, qsl,
                            start=True, stop=True,
                        )
                        if diag:
                            nc.vector.tensor_add(
                                ps_s[:, q0 : q0 + P], ps_s[:, q0 : q0 + P], mtri_sb[:]
                            )
                        es = espool.tile([P, 512], BF16, tag="es", name="es")
                        if not diag and FASTEXP_K and ndiag_ctr % FASTEXP_K == 0:
                            nc.vector.tensor_scalar(
                                es[:, q0:512].bitcast(I16), ps_s[:, q0:512],
                                FE_A, FE_B,
                                mybir.AluOpType.mult, mybir.AluOpType.add,
                            )
                        else:
                            nc.scalar.activation(
                                es[:, q0:512], ps_s[:, q0:512], AF.Exp, scale=SCALE
                            )
                        if not diag:
                            ndiag_ctr += 1
                        if QUAD_CS:
                            # DVE quad-merge: first tile of each quad has q0=0
                            # (full width); later diag tiles only touch their
                            # valid [q0:512] region, leaving earlier columns.
                            qi = idx % 4
                            if qi == 0:
                                qa = qapool.tile([P, 512], BF16, tag="qa", name="qa")
                                nc.vector.tensor_copy(qa[:], es[:])
                            else:
                                nc.vector.tensor_add(
                                    qa[:, q0:512], qa[:, q0:512], es[:, q0:512]
                                )
                            if qi == 3:
                                qd = idx // 4
                                nc.tensor.matmul(
                                    ps_cs[:], ones_sq[:], qa[:],
                                    start=(qd == 0), stop=(qd == n_quad - 1),
                                    skip_group_check=True,
                                )
                        while len(pend) >= max(PEND_DEPTH, 1):
                            flush_pend()
                        pend.append((ki, q0, es, blk, idx == 0, idx == n_kt_q - 1))
                    if h == 3 and qb == 3:
                        while pend:
                            flush_pend()

        # ---- Phase 3: o_proj (512 rows x 2048, full Wo from SBUF) ----
        # y rows 0:256 = my 256 rows of batch 0, rows 256:512 = of batch 1.
        # Row-tile r -> (batch r//2, row-half r%2). pass 1: heads h=0..2 of
        # each group (ready after the first three A2As) + bias -> y_sb (SBUF).
        # pass 2: h=3 only (the short post-A2A3 tail), added on DVE and
        # written out once on the sync ring.
        PASS1_T = [4 * g + hh for hh in range(3) for g in range(4)]
        PASS2_T = [4 * g + 3 for g in range(4)]
        with ExitStack() as ph3:
            ypool = ph3.enter_context(tc.tile_pool(name="yp", bufs=4))
            pso = ph3.enter_context(tc.tile_pool(name="pso", bufs=8, space="PSUM"))

            for nb in range(4):
                nsl = slice(512 * nb, 512 * (nb + 1))
                ps_os = [pso.tile([P, 512], F32, tag="po", name=f"ps_o{r}") for r in range(4)]
                for ti, t in enumerate(PASS1_T):
                    wsl = wo_sb[:, t, nsl]
                    for r in range(4):
                        nc.tensor.matmul(
                            ps_os[r][:], at[r // 2][t][:, P * (r % 2) : P * (r % 2 + 1)],
                            wsl, start=(ti == 0), stop=False,
                            skip_group_check=True,
                        )
                for r in range(4):
                    nc.tensor.matmul(
                        ps_os[r][:], ones_sq[0:1, :],
                        bo_sb[0:1, nsl], start=False, stop=True,
                        skip_group_check=True,
                    )
                    if YSB:
                        nc.vector.tensor_copy(y_sb[r][:, nsl], ps_os[r][:])
                    else:
                        ysb = ypool.tile([P, 512], BF16, tag="y", name="ysb")
                        nc.vector.tensor_copy(ysb[:], ps_os[r][:])
                        nc.sync.dma_start(y[r, nb, :, :], ysb[:])
            for nb in range(4):
                nsl = slice(512 * nb, 512 * (nb + 1))
                ps_o2 = [pso.tile([P, 512], F32, tag="po", name=f"ps_p{r}") for r in range(4)]
                for ti, t in enumerate(PASS2_T):
                    wsl = wo_sb[:, t, nsl]
                    for r in range(4):
                        nc.tensor.matmul(
                            ps_o2[r][:], at[r // 2][t][:, P * (r % 2) : P * (r % 2 + 1)],
                            wsl, start=(ti == 0), stop=(ti == len(PASS2_T) - 1),
                            skip_group_check=True,
                        )
                for r in range(4):
                    ysb = ypool.tile([P, 512], BF16, tag="y", name="ysb")
                    if YSB:
                        nc.vector.tensor_add(ysb[:], ps_o2[r][:], y_sb[r][:, nsl])
                        # alternate rings so the final writes drain in
                        # parallel instead of FIFO on one ring
                        ring = nc.sync if r % 2 == 0 else nc.scalar
                        ring.dma_start(y[r, nb, :, :], ysb[:])
                    else:
                        nc.vector.tensor_copy(ysb[:], ps_o2[r][:])
                        nc.gpsimd.dma_start(
                            y[r, nb, :, :], ysb[:],
                            accum_op=mybir.AluOpType.add,
                        )

    nc.compile()
    return nc


def _prelay_in(W):
    # [HID, C] -> on-chip [P, N_KT, C]: tile t holds rows 128t..128(t+1)
    C = W.shape[1]
    return np.ascontiguousarray(
        W.reshape(N_KT, P, C).transpose(1, 0, 2)
    ).astype(ml_dtypes.bfloat16)


def make_in_maps(hidden_states, Wq, bq, Wk, bk, Wv, bv, Wo, bo):
    hs = np.asarray(hidden_states, np.float32)
    # XT[b] pre-laid as [N_RB, P, N_KT, 512]: xt[rb, p, t, r] = X[b][512rb+r, 128t+p]
    XT = []
    for b in range(B):
        xb = hs[b].astype(ml_dtypes.bfloat16)  # [S, HID]
        XT.append(np.ascontiguousarray(
            xb.reshape(N_RB, 512, N_KT, P).transpose(0, 3, 2, 1)
        ))
    qq = np.arange(P)[None, :]
    kk = np.arange(P)[:, None]
    mtri = np.where(qq >= kk, 0.0, NEG).astype(np.float32)
    Wq = np.asarray(Wq, np.float32)
    Wk = np.asarray(Wk, np.float32)
    Wv = np.asarray(Wv, np.float32)
    Wo_b = _prelay_in(np.asarray(Wo, np.float32))
    bq = np.asarray(bq, np.float32)
    bk = np.asarray(bk, np.float32)
    bv = np.asarray(bv, np.float32)
    bo = np.asarray(bo, np.float32)
    in_maps = []
    for i in range(N_CORES):
        b, g = i // 4, i % 4
        in_maps.append({
            "xt": XT[b],
            "wq": _prelay_in(Wq[:, 512 * g : 512 * (g + 1)]),
            "wk": _prelay_in(Wk[:, 128 * g : 128 * (g + 1)]),
            "wv": _prelay_in(Wv[:, 128 * g : 128 * (g + 1)]),
            "bq": np.ascontiguousarray(bq[512 * g : 512 * (g + 1)]).reshape(512, 1),
            "bk": np.ascontiguousarray(bk[128 * g : 128 * (g + 1)]).reshape(128, 1),
            "bv": np.ascontiguousarray(bv[128 * g : 128 * (g + 1)]).reshape(128, 1),
            "wo": Wo_b,
            "bo": bo.reshape(1, HID).astype(ml_dtypes.bfloat16),
            "mtri": mtri,
            "onesd": np.ones((P, P), ml_dtypes.bfloat16),
        })
    return in_maps


def assemble(results):
    Y = np.empty((B, S, HID), np.float32)
    for i in range(N_CORES):
        yi = np.asarray(results[i]["y"]).astype(np.float32)
        yi = yi.transpose(0, 2, 1, 3).reshape(512, HID)
        Y[0, 256 * i : 256 * (i + 1), :] = yi[0:256]
        Y[1, 256 * i : 256 * (i + 1), :] = yi[256:512]
    return Y


_NC_CACHE = {}


def _get_nc(debug=False):
    if debug not in _NC_CACHE:
        _NC_CACHE[debug] = build_nc(debug=debug)
    return _NC_CACHE[debug]


def kernel(hidden_states, attention_mask, Wq, bq, Wk, bk, Wv, bv, Wo, bo):
    # attention_mask is all-ones for this problem (spec: fill=ones) -> ignored
    nc = _get_nc(debug=False)
    in_maps = make_in_maps(hidden_states, Wq, bq, Wk, bk, Wv, bv, Wo, bo)
    res = run_bass_kernel_spmd(nc, in_maps, core_ids=list(range(N_CORES)))
    return assemble(res.results)


# revision 18
# speedup vs baseline: 1.0371x; 1.0371x over previous
"""Trainium2 Bass kernel for causal GQA self-attention (B=2, S=2048, H=2048,
16 heads / 4 KV heads, head_dim 128) on 8 NeuronCores.

Sharding: core i = (batch b=i//4, kv-group g=i%4). Each core computes the QKV
projections for its batch restricted to its group's channels (q heads 4g..4g+3,
kv head g), attention for those 4 heads over its batch's 2048 rows, then a
8-rank AllToAll per head switches to row-sharding: core (b,g) computes rows
256*i of both batches through the full o_proj.

Per-core dataflow (channel-major layouts so no transposes of big activations):
  XT[h, r] (bf16, host-pretransposed) --matmul--> QT/KT/VT; V via
  dma_start_transpose of VT tiles; scores S^T[k, q] = KT_tile.T @ QT; causal
  handled by compile-time tile skipping + one [128,128] triangle mask; exp on
  ScalarE (scale folded in, every 2nd non-diag tile via DVE fast-exp bit
  trick); AV accumulated in PSUM over k tiles with a TWO-deep software
  pipeline interleaved across (h,qb) blocks so exp latency never bubbles the
  PE; softmax denominators via DVE quad-merge of exp tiles (3 adds per 4
  tiles) + ONE ones-matmul per quad (4x fewer PE colsum rows than per-tile);
  normalize with reciprocal_approx_fast + mul; per-head A2A (at-tile loads
  ride the scalar ring so they can never block the next head's ao writes);
  o_proj from SBUF-resident Wo, pass-1 (heads 0-2) results parked in SBUF
  bf16, pass-2 (head 3) added on DVE and written out once on the sync ring
  (no read-modify-write DMA tail).

DMA plan: xt stream entirely on sync ring with 6-chunk prefetch depth (the
ring sustains ~150GB/s when kept fed; splitting across rings let it idle);
QKV weights on scalar; consts + Wo bulk + collective triggers on gpsimd
(SWDGE hits 130-200GB/s for the 8MB Wo preload).
"""

import sys

sys.path.insert(0, "/opt/trn_rl_repo")

from collections import deque
from contextlib import ExitStack

import numpy as np
import ml_dtypes

import concourse.bass as bass
import concourse.mybir as mybir
import concourse.tile as tile
from concourse import bacc
from concourse.bass_utils import run_bass_kernel_spmd

F32 = mybir.dt.float32
BF16 = mybir.dt.bfloat16
I16 = mybir.dt.int16
AF = mybir.ActivationFunctionType

N_CORES = 8
B, S, HID = 2, 2048, 2048
NH, NKV, D = 16, 4, 128
SCALE = 1.0 / np.sqrt(D)
NEG = -1e30
P = 128
N_KT = HID // P  # 16 contraction tiles
N_RB = S // 512  # 4 row blocks (one batch per core)

import os
WO_PRELOAD = os.environ.get("K_WO_PRELOAD", "1") == "1"
# Every K-th non-diagonal exp tile runs as a 1-op DVE fast-exp (Schraudolph
# bf16 bit trick, ~2% rms) instead of on ScalarE; 0 disables.
FASTEXP_K = int(os.environ.get("K_FASTEXP", "3"))
# colsum via DVE quad-merge (1 ones-matmul per 4 exp tiles) vs per-tile
QUAD_CS = os.environ.get("K_QUAD", "1") == "1"
# AV software-pipeline depth (k-tiles the AV matmul trails the scores matmul)
PEND_DEPTH = int(os.environ.get("K_PEND", "6"))
# pass-1 o_proj parked in SBUF, pass-2 added on DVE (vs accum-DMA)
YSB = os.environ.get("K_YSB", "1") == "1"
LN2 = float(np.log(2.0))
FE_A = float(2.0**7 / LN2) * SCALE
FE_B = float(127.0 * 128.0 - 5.5)


def build_nc(debug=False):
    nc = bacc.Bacc("TRN2", target_bir_lowering=False, debug=debug, num_devices=8)

    # All big inputs are host-pre-permuted into their exact on-chip layouts so
    # every DMA is long-contiguous.
    xt = nc.dram_tensor("xt", [N_RB, P, N_KT, 512], BF16, kind="ExternalInput")
    wq = nc.dram_tensor("wq", [P, N_KT, 512], BF16, kind="ExternalInput")
    wk = nc.dram_tensor("wk", [P, N_KT, 128], BF16, kind="ExternalInput")
    wv = nc.dram_tensor("wv", [P, N_KT, 128], BF16, kind="ExternalInput")
    bq = nc.dram_tensor("bq", [512, 1], F32, kind="ExternalInput")
    bk = nc.dram_tensor("bk", [128, 1], F32, kind="ExternalInput")
    bv = nc.dram_tensor("bv", [128, 1], F32, kind="ExternalInput")
    wo = nc.dram_tensor("wo", [P, N_KT, HID], BF16, kind="ExternalInput")
    bo = nc.dram_tensor("bo", [1, HID], BF16, kind="ExternalInput")
    mtri = nc.dram_tensor("mtri", [P, P], F32, kind="ExternalInput")
    onesd = nc.dram_tensor("onesd", [P, P], BF16, kind="ExternalInput")
    y = nc.dram_tensor("y", [4, 4, P, 512], BF16, kind="ExternalOutput")

    with tile.TileContext(nc) as tc, ExitStack() as top:
        persist = top.enter_context(tc.tile_pool(name="persist", bufs=1))
        dram = top.enter_context(tc.tile_pool(name="dram", bufs=1, space="DRAM"))

        a2a_in = [dram.tile([8, P, 256], BF16, name=f"a2a_in{h}") for h in range(4)]
        a2a_out = [dram.tile([8, P, 256], BF16, name=f"a2a_out{h}") for h in range(4)]

        # Ring plan: sync = xt stream + ao writes + final y writes;
        # scalar = QKV weights + V transposes + at loads;
        # gpsimd = consts + Wo bulk + collective triggers.
        wq_sb = persist.tile([P, N_KT, 512], BF16, tag="wq")
        wk_sb = persist.tile([P, N_KT, 128], BF16, tag="wk")
        wv_sb = persist.tile([P, N_KT, 128], BF16, tag="wv")

        def load_w_chunk(kc):
            tsl = slice(4 * kc, 4 * (kc + 1))
            nc.scalar.dma_start(wq_sb[:, tsl, :], wq[:, tsl, :])
            nc.scalar.dma_start(wk_sb[:, tsl, :], wk[:, tsl, :])
            nc.scalar.dma_start(wv_sb[:, tsl, :], wv[:, tsl, :])

        load_w_chunk(0)

        ones_sq = persist.tile([P, P], BF16, tag="ones_sq")
        nc.gpsimd.dma_start(ones_sq[:], onesd[:])
        mtri_sb = persist.tile([P, P], F32, tag="mtri")
        nc.gpsimd.dma_start(mtri_sb[:], mtri[:])
        bq_sb = persist.tile([P, 4], F32, tag="bq")
        for h in range(4):
            nc.gpsimd.dma_start(bq_sb[:, h : h + 1], bq[128 * h : 128 * (h + 1), :])
        bk_sb = persist.tile([P, 1], F32, tag="bk")
        nc.gpsimd.dma_start(bk_sb[:], bk[:])
        bv_sb = persist.tile([P, 1], F32, tag="bv")
        nc.gpsimd.dma_start(bv_sb[:], bv[:])
        bo_sb = persist.tile([1, HID], BF16, tag="bo")
        nc.gpsimd.dma_start(bo_sb[:], bo[:])

        # Full Wo lives in SBUF: streamed on the gpsimd (SWDGE) ring, gated to
        # start only mid-phase-1 (see wo_gate below) so it can't starve the xt
        # stream; o_proj is then pure compute.
        wo_sb = persist.tile([P, N_KT, HID], BF16, tag="wo")

        # channel-major activations: partitions = feature dim
        qt_sb = [persist.tile([P, S], BF16, tag=f"qt{h}", name=f"qt{h}") for h in range(4)]
        kt_sb = persist.tile([P, S], BF16, tag="kt")
        v_sb = persist.tile([P, N_RB * 4, P], BF16, tag="v")  # [krow%128, ktile, d]
        vt_full = persist.tile([P, N_RB, 512], BF16, tag="vtf")
        if YSB:
            y_sb = [persist.tile([P, HID], BF16, tag=f"ysb{r}", name=f"ysb{r}") for r in range(4)]

        # ---- Phase 1: QKV projections (+ V transpose via DMA XBAR) ----
        with ExitStack() as ph1:
            xpool = ph1.enter_context(tc.tile_pool(name="xp", bufs=6))
            pspool = ph1.enter_context(tc.tile_pool(name="ps1", bufs=8, space="PSUM"))
            for rb in range(N_RB):
                rsl = slice(512 * rb, 512 * (rb + 1))
                # 4 chunks of 4 k-tiles each, all on the sync ring (weights own
                # the scalar ring); 6-buf pool keeps ~3 chunks prefetched ahead
                xt_c = [xpool.tile([P, 4, 512], BF16, tag="x", name=f"xt{rb}_{kc}") for kc in range(4)]
                for kc in range(4):
                    nc.sync.dma_start(xt_c[kc][:], xt[rb, :, 4 * kc : 4 * (kc + 1), :])
                    if rb == 0 and kc == 0:
                        load_w_chunk(1)
                    elif rb == 0 and kc == 2:
                        load_w_chunk(2)
                        load_w_chunk(3)
                if rb == 1 and WO_PRELOAD:
                    # gate the Wo stream on rb0's V evac, then emit its loads:
                    # they start once phase 1 is established and can't starve
                    # the early xt chunks.
                    wo_gate = persist.tile([1, 1], BF16, tag="wog")
                    nc.gpsimd.tensor_copy(wo_gate[:], vt_full[0:1, 0, 0:1])
                    for t in range(N_KT):
                        nc.gpsimd.dma_start(wo_sb[:, t, :], wo[:, t, :])
                ps_q = [pspool.tile([P, 512], F32, tag="ps1", name=f"ps_q{h}") for h in range(4)]
                ps_k = pspool.tile([P, 512], F32, tag="ps1", name="ps_k")
                ps_v = pspool.tile([P, 512], F32, tag="ps1", name="ps_v")
                for kt_i in range(N_KT):
                    st, sp = kt_i == 0, kt_i == N_KT - 1
                    x_sl = xt_c[kt_i // 4][:, kt_i % 4, :]
                    for h in range(4):
                        nc.tensor.matmul(
                            ps_q[h][:], wq_sb[:, kt_i, 128 * h : 128 * (h + 1)],
                            x_sl, start=st, stop=sp,
                        )
                    nc.tensor.matmul(ps_k[:], wk_sb[:, kt_i, :], x_sl, start=st, stop=sp)
                    nc.tensor.matmul(ps_v[:], wv_sb[:, kt_i, :], x_sl, start=st, stop=sp)
                for h in range(4):
                    nc.vector.tensor_scalar_add(qt_sb[h][:, rsl], ps_q[h][:], bq_sb[:, h : h + 1])
                nc.vector.tensor_scalar_add(kt_sb[:, rsl], ps_k[:], bk_sb[:])
                nc.vector.tensor_scalar_add(vt_full[:, rb, :], ps_v[:], bv_sb[:])
                # V transposes per rb, right after its evac: they interleave
                # behind the remaining weight chunks on the scalar ring and
                # finish before phase 2's first AV needs v_sb (instead of
                # all queueing in the phase-1 tail)
                for j in range(4):
                    m = 4 * rb + j
                    nc.scalar.dma_start_transpose(
                        v_sb[:, m, :], vt_full[:, rb, P * j : P * (j + 1)]
                    )

        # ---- Phase 2: attention (flash-style, S^T layout), A2A per head ----
        with ExitStack() as ph2:
            espool = ph2.enter_context(tc.tile_pool(name="es", bufs=PEND_DEPTH + 4))
            qapool = ph2.enter_context(tc.tile_pool(name="qa", bufs=2))
            bcpool = ph2.enter_context(tc.tile_pool(name="bc", bufs=2))
            aopool = ph2.enter_context(tc.tile_pool(name="ao", bufs=2))
            pss = ph2.enter_context(tc.tile_pool(name="pss", bufs=4, space="PSUM"))
            psav = ph2.enter_context(tc.tile_pool(name="psav", bufs=2, space="PSUM"))
            pscs = ph2.enter_context(tc.tile_pool(name="pscs", bufs=2, space="PSUM"))
            at = [
                [
                    persist.tile([P, 256], BF16, tag=f"at{bb}_{t}", name=f"at{bb}_{t}")
                    for t in range(N_KT)
                ]
                for bb in range(B)
            ]
            ndiag_ctr = 0
            # AV matmuls trail the scores matmuls by PEND_DEPTH k-tiles,
            # across (h,qb) block boundaries, so the exp of a block's last
            # tile overlaps the next block's first scores matmuls on the PE.
            # A block's normalization (recip/mul/ao DMA) and - for qb==3 -
            # its head's A2A trigger are emitted right after its final AV
            # flushes, keeping program order consistent with dataflow while
            # never parking the DVE on a not-yet-emitted matmul.
            pend = deque()

            def finalize_block(h, qb, ps_av, ps_cs):
                bc = bcpool.tile([P, 512], F32, tag="bc", name="bc")
                nc.vector.reciprocal_approx_fast(bc[:], ps_cs[:])
                ao = aopool.tile([P, 512], BF16, tag="ao", name="ao")
                nc.vector.tensor_mul(ao[:], ps_av[:], bc[:])
                nc.sync.dma_start(a2a_in[h][2 * qb, :, :], ao[:, 0:256])
                nc.sync.dma_start(a2a_in[h][2 * qb + 1, :, :], ao[:, 256:512])
                if qb == 3:
                    # ---- per-head 8-rank AllToAll; overlaps the next head's
                    # compute. Slot j carries my head's output for the 256
                    # rows of my batch that core j o_proj-owns.
                    nc.gpsimd.collective_compute(
                        "AllToAll",
                        mybir.AluOpType.bypass,
                        replica_groups=[list(range(N_CORES))],
                        ins=[a2a_in[h][:]],
                        outs=[a2a_out[h][:]],
                    )
                    # at-tile loads ride the GPSIMD ring: a dma_start's
                    # semaphore wait blocks the ISSUING engine, and gpsimd
                    # only carries cc triggers here - cc h+1 cannot start
                    # before cc h completes anyway (serial cc stream), so
                    # parking gpsimd on A2A-h completion costs nothing,
                    # while on sync/scalar it starved ao writes / exps.
                    for i in range(N_CORES):
                        bb, g = i // 4, i % 4
                        t = 4 * g + h
                        nc.gpsimd.dma_start(at[bb][t][:], a2a_out[h][i, :, :])

            def flush_pend():
                ki, q0, es, blk, st, sp = pend.popleft()
                h, qb, ps_av, ps_cs = blk
                nc.tensor.matmul(
                    ps_av[:, q0:512], v_sb[:, ki, :],
                    es[:, q0:512], start=st, stop=sp,
                    skip_group_check=True,
                )
                if not QUAD_CS:
                    nc.tensor.matmul(
                        ps_cs[:, q0:512], ones_sq[:],
                        es[:, q0:512], start=st, stop=sp,
                        skip_group_check=True,
                    )
                if sp:
                    finalize_block(h, qb, ps_av, ps_cs)

            for h in range(4):
                for qb in range(4):
                    # diagonal k-tiles first (full q width on the first)
                    ktiles = list(range(4 * qb, 4 * qb + 4)) + list(range(4 * qb))
                    ps_av = psav.tile([P, 512], F32, tag="av", name="ps_av")
                    ps_cs = pscs.tile([P, 512], F32, tag="cs", name="ps_cs")
                    blk = (h, qb, ps_av, ps_cs)
                    n_kt_q = len(ktiles)
                    n_quad = n_kt_q // 4
                    qa = None
                    for idx, ki in enumerate(ktiles):
                        diag = ki >= 4 * qb
                        q0 = 128 * ki - 512 * qb if diag else 0
                        ps_s = pss.tile([P, 512], F32, tag="s", name="ps_s")
                        ksl = kt_sb[:, P * ki : P * (ki + 1)]
                        qsl = qt_sb[h][:, 512 * qb + q0 : 512 * (qb + 1)]
                        nc.tensor.matmul(
                            ps_s[:, q0:512], ksl, qsl,
                            start=True, stop=True,
                        )
                        if diag:
                            nc.vector.tensor_add(
                                ps_s[:, q0 : q0 + P], ps_s[:, q0 : q0 + P], mtri_sb[:]
                            )
                        es = espool.tile([P, 512], BF16, tag="es", name="es")
                        if not diag and FASTEXP_K and ndiag_ctr % FASTEXP_K == 0:
                            nc.vector.tensor_scalar(
                                es[:, q0:512].bitcast(I16), ps_s[:, q0:512],
                                FE_A, FE_B,
                                mybir.AluOpType.mult, mybir.AluOpType.add,
                            )
                        else:
                            nc.scalar.activation(
                                es[:, q0:512], ps_s[:, q0:512], AF.Exp, scale=SCALE
                            )
                        if not diag:
                            ndiag_ctr += 1
                        if QUAD_CS:
                            # DVE quad-merge: first tile of each quad has q0=0
                            # (full width); later diag tiles only touch their
                            # valid [q0:512] region, leaving earlier columns.
                            qi = idx % 4
                            if qi == 0:
                                qa = qapool.tile([P, 512], BF16, tag="qa", name="qa")
                                nc.vector.tensor_copy(qa[:], es[:])
                            else:
                                nc.vector.tensor_add(
                                    qa[:, q0:512], qa[:, q0:512], es[:, q0:512]
                                )
                            if qi == 3:
                                qd = idx // 4
                                nc.tensor.matmul(
                                    ps_cs[:], ones_sq[:], qa[:],
                                    start=(qd == 0), stop=(qd == n_quad - 1),
                                    skip_group_check=True,
                                )
                        while len(pend) >= max(PEND_DEPTH, 1):
                            flush_pend()
                        pend.append((ki, q0, es, blk, idx == 0, idx == n_kt_q - 1))
                    if h == 3 and qb == 3:
                        while pend:
                            flush_pend()

        # ---- Phase 3: o_proj (512 rows x 2048, full Wo from SBUF) ----
        # y rows 0:256 = my 256 rows of batch 0, rows 256:512 = of batch 1.
        # Row-tile r -> (batch r//2, row-half r%2). pass 1: heads h=0..2 of
        # each group (ready after the first three A2As) + bias -> y_sb (SBUF).
        # pass 2: h=3 only (the short post-A2A3 tail), added on DVE and
        # written out once on the sync ring.
        PASS1_T = [4 * g + hh for hh in range(3) for g in range(4)]
        PASS2_T = [4 * g + 3 for g in range(4)]
        with ExitStack() as ph3:
            ypool = ph3.enter_context(tc.tile_pool(name="yp", bufs=4))
            pso = ph3.enter_context(tc.tile_pool(name="pso", bufs=8, space="PSUM"))

            for nb in range(4):
                nsl = slice(512 * nb, 512 * (nb + 1))
                ps_os = [pso.tile([P, 512], F32, tag="po", name=f"ps_o{r}") for r in range(4)]
                for ti, t in enumerate(PASS1_T):
                    wsl = wo_sb[:, t, nsl]
                    for r in range(4):
                        nc.tensor.matmul(
                            ps_os[r][:], at[r // 2][t][:, P * (r % 2) : P * (r % 2 + 1)],
                            wsl, start=(ti == 0), stop=False,
                            skip_group_check=True,
                        )
                for r in range(4):
                    nc.tensor.matmul(
                        ps_os[r][:], ones_sq[0:1, :],
                        bo_sb[0:1, nsl], start=False, stop=True,
                        skip_group_check=True,
                    )
                    if YSB:
                        nc.vector.tensor_copy(y_sb[r][:, nsl], ps_os[r][:])
                    else:
                        ysb = ypool.tile([P, 512], BF16, tag="y", name="ysb")
                        nc.vector.tensor_copy(ysb[:], ps_os[r][:])
                        nc.sync.dma_start(y[r, nb, :, :], ysb[:])
            for nb in range(4):
                nsl = slice(512 * nb, 512 * (nb + 1))
                ps_o2 = [pso.tile([P, 512], F32, tag="po", name=f"ps_p{r}") for r in range(4)]
                for ti, t in enumerate(PASS2_T):
                    wsl = wo_sb[:, t, nsl]
                    for r in range(4):
                        nc.tensor.matmul(
                            ps_o2[r][:], at[r // 2][t][:, P * (r % 2) : P * (r % 2 + 1)],
                            wsl, start=(ti == 0), stop=(ti == len(PASS2_T) - 1),
                            skip_group_check=True,
                        )
                for r in range(4):
                    ysb = ypool.tile([P, 512], BF16, tag="y", name="ysb")
                    if YSB:
                        nc.vector.tensor_add(ysb[:], ps_o2[r][:], y_sb[r][:, nsl])
                        nc.sync.dma_start(y[r, nb, :, :], ysb[:])
                    else:
                        nc.vector.tensor_copy(ysb[:], ps_o2[r][:])
                        nc.gpsimd.dma_start(
                            y[r, nb, :, :], ysb[:],
                            accum_op=mybir.AluOpType.add,
                        )

    nc.compile()
    return nc


def _prelay_in(W):
    # [HID, C] -> on-chip [P, N_KT, C]: tile t holds rows 128t..128(t+1)
    C = W.shape[1]
    return np.ascontiguousarray(
        W.reshape(N_KT, P, C).transpose(1, 0, 2)
    ).astype(ml_dtypes.bfloat16)


def make_in_maps(hidden_states, Wq, bq, Wk, bk, Wv, bv, Wo, bo):
    hs = np.asarray(hidden_states, np.float32)
    # XT[b] pre-laid as [N_RB, P, N_KT, 512]: xt[rb, p, t, r] = X[b][512rb+r, 128t+p]
    XT = []
    for b in range(B):
        xb = hs[b].astype(ml_dtypes.bfloat16)  # [S, HID]
        XT.append(np.ascontiguousarray(
            xb.reshape(N_RB, 512, N_KT, P).transpose(0, 3, 2, 1)
        ))
    qq = np.arange(P)[None, :]
    kk = np.arange(P)[:, None]
    mtri = np.where(qq >= kk, 0.0, NEG).astype(np.float32)
    Wq = np.asarray(Wq, np.float32)
    Wk = np.asarray(Wk, np.float32)
    Wv = np.asarray(Wv, np.float32)
    Wo_b = _prelay_in(np.asarray(Wo, np.float32))
    bq = np.asarray(bq, np.float32)
    bk = np.asarray(bk, np.float32)
    bv = np.asarray(bv, np.float32)
    bo = np.asarray(bo, np.float32)
    in_maps = []
    for i in range(N_CORES):
        b, g = i // 4, i % 4
        in_maps.append({
            "xt": XT[b],
            "wq": _prelay_in(Wq[:, 512 * g : 512 * (g + 1)]),
            "wk": _prelay_in(Wk[:, 128 * g : 128 * (g + 1)]),
            "wv": _prelay_in(Wv[:, 128 * g : 128 * (g + 1)]),
            "bq": np.ascontiguousarray(bq[512 * g : 512 * (g + 1)]).reshape(512, 1),
            "bk": np.ascontiguousarray(bk[128 * g : 128 * (g + 1)]).reshape(128, 1),
            "bv": np.ascontiguousarray(bv[128 * g : 128 * (g + 1)]).reshape(128, 1),
            "wo": Wo_b,
            "bo": bo.reshape(1, HID).astype(ml_dtypes.bfloat16),
            "mtri": mtri,
            "onesd": np.ones((P, P), ml_dtypes.bfloat16),
        })
    return in_maps


def assemble(results):
    Y = np.empty((B, S, HID), np.float32)
    for i in range(N_CORES):
        yi = np.asarray(results[i]["y"]).astype(np.float32)
        yi = yi.transpose(0, 2, 1, 3).reshape(512, HID)
        Y[0, 256 * i : 256 * (i + 1), :] = yi[0:256]
        Y[1, 256 * i : 256 * (i + 1), :] = yi[256:512]
    return Y


_NC_CACHE = {}


def _get_nc(debug=False):
    if debug not in _NC_CACHE:
        _NC_CACHE[debug] = build_nc(debug=debug)
    return _NC_CACHE[debug]


def kernel(hidden_states, attention_mask, Wq, bq, Wk, bk, Wv, bv, Wo, bo):
    # attention_mask is all-ones for this problem (spec: fill=ones) -> ignored
    nc = _get_nc(debug=False)
    in_maps = make_in_maps(hidden_states, Wq, bq, Wk, bk, Wv, bv, Wo, bo)
    res = run_bass_kernel_spmd(nc, in_maps, core_ids=list(range(N_CORES)))
    return assemble(res.results)
